# revision 4
# baseline (speedup 1.0000x reference)
"""Bass/Trainium2 kernel for nn_BiencoderRanker: pairwise cosine similarity.

scores[n, m] = <pred_n, cand_m> / (|pred_n| * |cand_m|)
  fp_pred: (1024, 4096) fp32, fp_cand: (16384, 4096) fp32 -> scores (1024, 16384) fp32

Sharding: fp_cand split along M across 8 cores (2048 rows each); fp_pred
replicated. Each core computes its (1024, 2048) tile; host concatenates.

v2 (fp8 DoubleRow GEMM), ~4x PE throughput vs the bf16 v1:
- Host marshalling: fp32 -> fp8e4m3 (RNE), uploaded twice per core: K-major
  tiled copies for the matmul path (kills all device-side xbar transposes)
  and natural-layout copies for the norm path. The device then computes
  cos(quantized a, quantized b) exactly up to fp32 psum accumulation and a
  single bf16 output rounding -- ~0.5% worst-case vs the fp32 reference.
- Matmul: PE fp8 DoubleRow (two 128-deep k-slices per instruction, 0.5
  cyc/moving-row): psum[n128, m512] accumulates over 16 K=256 chunks.
- Norms: natural fp8 row-chunks; square+row-sum split across ACT (fused
  Square+accum_out), Pool and DVE (square tensor_tensor then tensor_reduce);
  sqrt on ACT, reciprocal on DVE (Rsqrt is banned as inaccurate).
- Join: ACT drains psum with scale=1/|pred_n| (per-partition Copy-mul)
  to fp32; DVE multiplies by the broadcast 1/|cand_m| row writing bf16.
  1/|cand| still round-trips through DRAM onto the free axis.
"""

import numpy as np
import ml_dtypes

import concourse.bacc as bacc
import concourse.mybir as mybir
import concourse.tile as tile
from concourse.bass_utils import run_bass_kernel_spmd

P = 128
N = 1024  # fp_pred rows
K = 4096  # feature dim
M_FULL = 16384  # fp_cand rows
N_CORES = 8
M = M_FULL // N_CORES  # cand rows per core
NB = N // P  # 8 pred row-chunks
MBLK = 4  # m-blocks per core (512 cand rows each)
MC = 4  # 128-row chunks per m-block
KC = K // P  # 32 contraction chunks of 128
KP = KC // 2  # 16 DoubleRow chunks of 256
FREE = 512  # matmul moving free dim / psum bank width

F32 = mybir.dt.float32
BF16 = mybir.dt.bfloat16
F8 = mybir.dt.float8e4
AF = mybir.ActivationFunctionType
NP_F8 = ml_dtypes.float8_e4m3

# norm-chunk engine assignment: "a" = ACT fused square+accum (1 pass),
# "p"/"d" = Pool/DVE square then reduce (2 passes). Balanced so each
# engine's total (incl. ACT drains + DVE output scaling) stays under the
# DMA-bound envelope.
PRED_ENG = ["a", "p", "d", "a", "p", "d", "a", "p"]
CAND_ENG = [
    ["a", "a", "p", "d"],
    ["a", "a", "p", "d"],
    ["a", "a", "p", "d"],
    ["a", "a", "p", "p"],
]

_compiled = None


def _build(repeats=1):
    nc = bacc.Bacc(None, target_bir_lowering=False)
    # matmul-path (K-major, grouped so every DMA line is contiguous per
    # partition): pred_t[p, nb, kc, q] = pred[nb*128+q, kc*128+p]
    pred_t = nc.dram_tensor("pred_t", (P, NB, KC, P), F8, kind="ExternalInput")
    # cand_t[p, mb, kc, ms] = cand[mb*512+ms, kc*128+p]  (per-core rows)
    cand_t = nc.dram_tensor("cand_t", (P, MBLK, KC, FREE), F8, kind="ExternalInput")
    # norm-path natural layouts
    pred_nat = nc.dram_tensor("pred_nat", (N, K), F8, kind="ExternalInput")
    cand_nat = nc.dram_tensor("cand_nat", (M, K), F8, kind="ExternalInput")
    out = nc.dram_tensor("scores", (N, M), BF16, kind="ExternalOutput")

    with tile.TileContext(nc) as tc:
        with (
            tc.tile_pool(name="dram", bufs=1, space="DRAM") as dram_pool,
            tc.tile_pool(name="at", bufs=1) as at_pool,
            tc.tile_pool(name="bt", bufs=2) as bt_pool,
            tc.tile_pool(name="stage", bufs=4) as stage_pool,
            tc.tile_pool(name="sq", bufs=2) as sq_pool,
            tc.tile_pool(name="norm", bufs=8) as norm_pool,
            tc.tile_pool(name="invs", bufs=2) as inv_pool,
            tc.tile_pool(name="outf", bufs=10) as outf_pool,
            tc.tile_pool(name="outb", bufs=6) as outb_pool,
            tc.tile_pool(name="psum", bufs=6, space="PSUM") as psum_pool,
        ):
            # staging to move 1/|cand| from the partition axis to the free axis
            invb_dram = dram_pool.tile([M], F32, name="invb_rt")
            aT = at_pool.tile([P, NB, KC, P], F8, name="aT", bufs=1)
            inv_a = norm_pool.tile([P, NB], F32, bufs=1, name="inv_a")
            bts = {}

            def bt_tile(mb):
                if mb not in bts:
                    bts[mb] = bt_pool.tile([P, KC, FREE], F8, tag="bt", name=f"bT{mb}")
                return bts[mb]

            def t_a(nb):
                nc.sync.dma_start(aT[:, nb], pred_t[:, nb])

            def t_b(mb):
                bT = bt_tile(mb)
                for s in range(4):  # 4 sub-DMAs so the PE can start early
                    nc.sync.dma_start(
                        bT[:, 8 * s : 8 * (s + 1), :],
                        cand_t[:, mb, 8 * s : 8 * (s + 1), :],
                    )

            # ---- norm path ----
            def norm_chunk(dram_rows, inv_dst, idx, eng):
                """inv_dst [P, 1] <- 1/|row| for 128 fp8 rows."""
                nat = stage_pool.tile([P, K], F8, tag="stage", name=f"nat{idx}")
                nc.gpsimd.dma_start(nat[:], dram_rows)
                ssq = norm_pool.tile([P, 1], F32, tag="norm", name=f"ssq{idx}")
                if eng == "a":
                    sq = sq_pool.tile([P, K], BF16, tag="sqa", name=f"sqa{idx}", bufs=1)
                    nc.scalar.activation(sq[:], nat[:], AF.Square, accum_out=ssq[:])
                elif eng == "d":
                    sq = sq_pool.tile([P, K], BF16, tag="sqd", name=f"sqd{idx}", bufs=2)
                    nc.vector.tensor_tensor(sq[:], nat[:], nat[:], mybir.AluOpType.mult)
                    nc.vector.tensor_reduce(
                        ssq[:], sq[:], mybir.AxisListType.X, mybir.AluOpType.add
                    )
                else:
                    # Pool can't free-axis reduce: square, then fold halves
                    # in place down to [P, 32]; DVE finishes the tiny reduce.
                    sq = sq_pool.tile([P, K], F32, tag="sqp", name=f"sqp{idx}", bufs=2)
                    nc.gpsimd.tensor_tensor(sq[:], nat[:], nat[:], mybir.AluOpType.mult)
                    w = K // 2
                    while w >= 32:
                        nc.gpsimd.tensor_tensor(
                            sq[:, :w], sq[:, :w], sq[:, w : 2 * w], mybir.AluOpType.add
                        )
                        w //= 2
                    nc.vector.tensor_reduce(
                        ssq[:], sq[:, :32], mybir.AxisListType.X, mybir.AluOpType.add
                    )
                nrm = norm_pool.tile([P, 1], F32, tag="norm", name=f"nrm{idx}")
                nc.scalar.activation(nrm[:], ssq[:], AF.Sqrt)
                nc.vector.reciprocal(inv_dst, nrm[:])

            def norm_a(nb):
                norm_chunk(
                    pred_nat[nb * P : (nb + 1) * P, :],
                    inv_a[:, nb : nb + 1],
                    f"a{nb}",
                    PRED_ENG[nb],
                )

            invbs = {}

            def norm_b(mb, mc):
                if mb not in invbs:
                    invbs[mb] = inv_pool.tile([P, MC], F32, tag="invb", name=f"invb{mb}")
                r0 = (mb * MC + mc) * P
                norm_chunk(
                    cand_nat[r0 : r0 + P, :],
                    invbs[mb][:, mc : mc + 1],
                    f"b{mb}_{mc}",
                    CAND_ENG[mb][mc],
                )

            def invb_roundtrip(mb):
                # scatter [P, MC] -> invb_rt[mb*512 + mc*128 + p], reload as a
                # row, broadcast across partitions.
                nc.scalar.dma_start(
                    invb_dram[mb * FREE : (mb + 1) * FREE].rearrange(
                        "(mc p) -> p mc", p=P
                    ),
                    invbs[mb][:],
                )
                row = inv_pool.tile([1, FREE], F32, tag="invrow", name=f"invrow{mb}")
                nc.scalar.dma_start(
                    row[:], invb_dram[None, mb * FREE : (mb + 1) * FREE]
                )
                bcast = inv_pool.tile([P, FREE], F32, tag="invbc", name=f"invbc{mb}")
                nc.gpsimd.partition_broadcast(bcast[:], row[:])
                return bcast

            # ---- matmul path ----
            def mm_block(mb, inv_bcast):
                bT = bts[mb]
                for nb in range(NB):
                    ps = psum_pool.tile(
                        [P, FREE], F32, tag="ps", name=f"ps{mb}_{nb}", bufs=6
                    )
                    for j in range(KP):
                        nc.tensor.matmul(
                            ps[:],
                            aT[:, nb, 2 * j : 2 * j + 2, :],
                            bT[:, 2 * j : 2 * j + 2, :],
                            start=(j == 0),
                            stop=(j == KP - 1),
                            perf_mode=mybir.MatmulPerfMode.DoubleRow,
                        )
                    # drain + per-partition 1/|pred| scale in one ACT pass
                    ot = outf_pool.tile([P, FREE], F32, tag="otf", name=f"ot{mb}_{nb}")
                    nc.scalar.mul(ot[:], ps[:], inv_a[:, nb : nb + 1])
                    ob = outb_pool.tile([P, FREE], BF16, tag="otb", name=f"ob{mb}_{nb}")
                    nc.vector.tensor_tensor(
                        ob[:], ot[:], inv_bcast[:], mybir.AluOpType.mult
                    )
                    nc.scalar.dma_start(
                        out[nb * P : (nb + 1) * P, mb * FREE : (mb + 1) * FREE],
                        ob[:],
                    )

            # ---- emission (scheduler reorders per-engine by readiness) ----
            for _rep in range(repeats):
                bts.clear()
                invbs.clear()

                t_a(0)
                t_b(0)
                for nb in range(1, NB):
                    t_a(nb)
                for nb in range(NB):
                    norm_a(nb)
                for mb in range(MBLK):
                    if mb + 1 < MBLK:
                        t_b(mb + 1)
                    for mc in range(MC):
                        norm_b(mb, mc)
                    invbc = invb_roundtrip(mb)
                    mm_block(mb, invbc)
    nc.compile()
    return nc


def _get_compiled():
    global _compiled
    if _compiled is None:
        _compiled = _build()
    return _compiled


def _in_maps(fp_pred: np.ndarray, fp_cand: np.ndarray) -> list[dict]:
    """Host marshalling: fp8 cast + K-major tiled copies, per core."""
    pred_f8 = np.asarray(fp_pred, dtype=np.float32).astype(NP_F8)
    cand_f8 = np.asarray(fp_cand, dtype=np.float32).astype(NP_F8)
    # pred_t[p, nb, kc, q] = pred[nb*128+q, kc*128+p]
    pred_t = np.ascontiguousarray(
        pred_f8.reshape(NB, P, KC, P).transpose(3, 0, 2, 1)
    )
    maps = []
    for i in range(N_CORES):
        cshard = np.ascontiguousarray(cand_f8[i * M : (i + 1) * M])
        cand_t = np.ascontiguousarray(
            cshard.reshape(MBLK, FREE, KC, P).transpose(3, 0, 2, 1)
        )
        maps.append(
            {
                "pred_t": pred_t,
                "cand_t": cand_t,
                "pred_nat": pred_f8,
                "cand_nat": cshard,
            }
        )
    return maps


def kernel(fp_pred: np.ndarray, fp_cand: np.ndarray) -> np.ndarray:
    fp_pred = np.asarray(fp_pred, dtype=np.float32)
    fp_cand = np.asarray(fp_cand, dtype=np.float32)
    assert fp_pred.shape == (N, K) and fp_cand.shape == (M_FULL, K)

    nc = _get_compiled()
    res = run_bass_kernel_spmd(nc, _in_maps(fp_pred, fp_cand), core_ids=list(range(N_CORES)))
    return np.concatenate(
        [res.results[i]["scores"].astype(np.float32) for i in range(N_CORES)], axis=1
    )
